# revision 8
# baseline (speedup 1.0000x reference)
"""Trainium2 Bass kernel for nn_BasicConvolutionBlock (sparse conv + BN + ReLU).

Math (per reference):
    conv[n] = sum_k feats[nbr_idx[n, k]] @ W[k]       # [N, 96], k = 0..26
    y = (conv - mean) * rsqrt(var + eps) * gamma + beta ; relu(y)

Distribution: voxel (N) dimension sharded across 8 NeuronCores; feats table
(bf16 — halves the random-gather DMA bytes vs f32; rel err ~4e-3, well under
the 2e-2 gate) and weights replicated per core.

Two NEFFs (the AllReduce-in-kernel path is unstable under the axon PJRT
bridge, so per-core BN partial sums are combined on the host -- 768 B of
float math -- between the two device passes):

  pass 1 (per core, per 512-voxel tile):
    - 108 indirect DMA gathers (128 bf16 rows of 192 B each) -> SBUF
      [128, 108, 96].  One indirect DMA per idx column: multi-column offset
      APs return garbage on HW (the SWDGE ucode only honors [128, 1]
      offsets), so the ~1 us/instruction SWDGE overhead is the pass-1 floor.
    - per group of 3 offsets k: 12 PE transposes (bf16: 1 cyc/row)
      [128v, 96c] -> PSUM [96, 1536] bf16, one PSUM->SBUF copy (alternating
      DVE / scalar engine to balance load), 3 accumulating bf16 matmuls
      W_k.T @ G_k.T -> PSUM [96, 512] f32
    - BN partial sum/sumsq via ACT accum; conv staged to DRAM channel-major
      in bf16
  pass 2: y = relu(conv * scale + shift) (fused ACT op, f32 out), PE
    transpose back to row-major, store f32.
"""
import numpy as np

import concourse.bass as bass
import concourse.bacc as bacc
import concourse.tile as tile
import concourse.mybir as mybir
from concourse.masks import make_identity

F32 = mybir.dt.float32
BF16 = mybir.dt.bfloat16
I32 = mybir.dt.int32
AF = mybir.ActivationFunctionType
BF16_NP = mybir.dt.np(BF16)

N_TOTAL = 262144
C = 96
KVOL = 27
N_CORES = 8
N_PER_CORE = N_TOTAL // N_CORES      # 32768
TILE_V = 512                         # voxels per tile
BLOCKS = TILE_V // 128               # 4
J = KVOL * BLOCKS                    # 108 gathered rows per partition per tile
N_TILES = N_PER_CORE // TILE_V       # 64
BN_EPS = 1e-5
KGRP = 3
CP = 128                             # channel-padded row (256 B bf16)
NWIN = 8
WROWS = N_TOTAL // NWIN              # 32768
NREQ = KVOL * TILE_V                 # 13824
WSLOTS = 2048
I16 = mybir.dt.int16


_cache = {}


# --------------------------------------------------------------------------
# graph builders
# --------------------------------------------------------------------------
def build_pass1(n_tiles=N_TILES, n_cores=N_CORES, gbufs=2, rbufs=3):
    nc = bacc.Bacc("TRN2", target_bir_lowering=False, debug=False,
                   num_devices=n_cores)

    featsb = nc.dram_tensor("featsb", [N_TOTAL, C], BF16,
                            kind="ExternalInput")
    # host-arranged: idx_arr[t, p, k*4+b] = nbr[t*512 + b*128 + p, k]
    idx_arr = nc.dram_tensor("idx_arr", [n_tiles, 128, J], I32,
                             kind="ExternalInput")
    # host-arranged: Wb[ci, k, co] = W[k, ci, co] in bf16
    Wb = nc.dram_tensor("Wb", [C, KVOL, C], BF16, kind="ExternalInput")
    convT = nc.dram_tensor("convT", [C, n_tiles * TILE_V], BF16,
                           kind="ExternalOutput")
    stats = nc.dram_tensor("stats", [C, 2], F32, kind="ExternalOutput")

    with tile.TileContext(nc) as tc:
        with (
            tc.tile_pool(name="const", bufs=1) as cpool,
            tc.tile_pool(name="gp", bufs=gbufs) as gpool,
            tc.tile_pool(name="rp", bufs=rbufs) as rpool,
            tc.tile_pool(name="sp", bufs=2) as spool,
            tc.tile_pool(name="psA", bufs=2, space="PSUM") as psA,
            tc.tile_pool(name="psB", bufs=2, space="PSUM") as psB,
        ):
            ident = cpool.tile([128, 128], BF16, tag="ident")
            make_identity(nc, ident[:])
            w_sb = cpool.tile([C, KVOL, C], BF16, tag="w")
            nc.sync.dma_start(w_sb[:], Wb[:])

            sum_acc = cpool.tile([C, n_tiles], F32, tag="sum_acc")
            sq_acc = cpool.tile([C, n_tiles], F32, tag="sq_acc")

            for t in range(n_tiles):
                idx_sb = gpool.tile([128, J], I32, tag="idx")
                nc.sync.dma_start(idx_sb[:], idx_arr[t, :, :])

                g_sb = gpool.tile([128, J, C], BF16, tag="g")
                # NOTE: one indirect DMA per column — multi-column offset
                # APs return garbage on HW (ucode only honors [128, 1]).
                for j in range(J):
                    nc.gpsimd.indirect_dma_start(
                        out=g_sb[:, j, :],
                        out_offset=None,
                        in_=featsb[:],
                        in_offset=bass.IndirectOffsetOnAxis(
                            ap=idx_sb[:, j:j + 1], axis=0),
                    )

                out_ps = psA.tile([C, TILE_V], F32, tag="outp")
                for gi, k0 in enumerate(range(0, KVOL, KGRP)):
                    tp = psB.tile([C, KGRP * TILE_V], BF16, tag="tp")
                    for dk in range(KGRP):
                        for b in range(BLOCKS):
                            nc.tensor.transpose(
                                tp[:, (dk * BLOCKS + b) * 128:
                                   (dk * BLOCKS + b + 1) * 128],
                                g_sb[:, (k0 + dk) * BLOCKS + b, :],
                                ident[:],
                            )
                    rhs = rpool.tile([C, KGRP * TILE_V], BF16, tag="rhs")
                    if gi % 2 == 0:
                        nc.vector.tensor_copy(rhs[:], tp[:])
                    else:
                        nc.scalar.activation(rhs[:], tp[:], AF.Identity)
                    for dk in range(KGRP):
                        k = k0 + dk
                        nc.tensor.matmul(
                            out_ps[:], w_sb[:, k, :],
                            rhs[:, dk * TILE_V:(dk + 1) * TILE_V],
                            start=(k == 0), stop=(k == KVOL - 1),
                        )

                conv_sb = spool.tile([C, TILE_V], BF16, tag="conv")
                nc.scalar.activation(
                    conv_sb[:], out_ps[:], AF.Identity,
                    accum_out=sum_acc[:, t:t + 1])
                sq_sb = spool.tile([C, TILE_V], BF16, tag="sq")
                nc.scalar.activation(
                    sq_sb[:], conv_sb[:], AF.Square,
                    accum_out=sq_acc[:, t:t + 1])
                nc.sync.dma_start(
                    convT[:, t * TILE_V:(t + 1) * TILE_V], conv_sb[:])

            stats_sb = cpool.tile([C, 2], F32, tag="stats_sb")
            nc.vector.reduce_sum(
                stats_sb[:, 0:1], sum_acc[:], axis=mybir.AxisListType.X)
            nc.vector.reduce_sum(
                stats_sb[:, 1:2], sq_acc[:], axis=mybir.AxisListType.X)
            nc.sync.dma_start(stats[:], stats_sb[:])

    nc.finalize()
    return nc



def build_pass1_v2(n_tiles=N_TILES, n_cores=N_CORES, gbufs=2):
    from concourse import library_config
    nc = bacc.Bacc("TRN2", target_bir_lowering=False, debug=False,
                   num_devices=n_cores)

    featsP = nc.dram_tensor("featsP", [N_TOTAL, CP], BF16,
                            kind="ExternalInput")
    l1idx = nc.dram_tensor("l1idx", [n_tiles, NWIN, 128, WSLOTS // 16], I16,
                           kind="ExternalInput")
    l2idx = nc.dram_tensor("l2idx", [n_tiles, 128, NREQ // 16], I16,
                           kind="ExternalInput")
    Wb = nc.dram_tensor("Wb", [CP, KVOL, C], BF16, kind="ExternalInput")
    convT = nc.dram_tensor("convT", [C, n_tiles * TILE_V], BF16,
                           kind="ExternalOutput")
    stats = nc.dram_tensor("stats", [C, 2], F32, kind="ExternalOutput")

    GCALL = 1024                       # max idxs per dma_gather call
    with tile.TileContext(nc) as tc:
        with (
            tc.tile_pool(name="const", bufs=1) as cpool,
            tc.tile_pool(name="gp", bufs=gbufs) as gpool,
            tc.tile_pool(name="sp", bufs=2) as spool,
            tc.tile_pool(name="psA", bufs=2, space="PSUM") as psA,
        ):
            nc.gpsimd.load_library(library_config.mlp)
            w_sb = cpool.tile([CP, KVOL, C], BF16, tag="w")
            nc.sync.dma_start(w_sb[:], Wb[:])
            sum_acc = cpool.tile([C, n_tiles], F32, tag="sum_acc")
            sq_acc = cpool.tile([C, n_tiles], F32, tag="sq_acc")

            for t in range(n_tiles):
                i1 = gpool.tile([128, NWIN, WSLOTS // 16], I16, tag="i1")
                nc.sync.dma_start(i1[:], l1idx[t].rearrange("w p s -> p w s"))
                i2 = gpool.tile([128, NREQ // 16], I16, tag="i2")
                nc.sync.dma_start(i2[:], l2idx[t])

                stag = gpool.tile([128, NWIN, WSLOTS // 128, CP], BF16,
                                  tag="stag")
                for w in range(NWIN):
                    for h in range(WSLOTS // GCALL):
                        r0 = h * (GCALL // 128)
                        nc.gpsimd.dma_gather(
                            stag[:, w, r0:r0 + GCALL // 128, :],
                            featsP[w * WROWS:(w + 1) * WROWS, :],
                            i1[:, w, h * (GCALL // 16):(h + 1) * (GCALL // 16)],
                            GCALL, GCALL, CP,
                            transpose=False)

                rhs = gpool.tile([128, 1, NREQ], BF16, tag="rhs")
                for h in range(NREQ // TILE_V):
                    nc.gpsimd.dma_gather(
                        rhs[:, :, h * TILE_V:(h + 1) * TILE_V],
                        stag[:].rearrange("p w r e -> p (w r e)"),
                        i2[:, h * (TILE_V // 16):(h + 1) * (TILE_V // 16)],
                        TILE_V, TILE_V, CP,
                        transpose=True,
                        sbuf_tokens_per_rank=128,
                        sbuf_free_dim_per_rank=CP * 2,
                        sbuf_free_dim_pad_per_rank=0,
                        sbuf_byte_offset=0)

                out_ps = psA.tile([C, TILE_V], F32, tag="outp")
                for k in range(KVOL):
                    nc.tensor.matmul(
                        out_ps[:], w_sb[:, k, :],
                        rhs[:, 0, k * TILE_V:(k + 1) * TILE_V],
                        start=(k == 0), stop=(k == KVOL - 1),
                    )

                conv_sb = spool.tile([C, TILE_V], BF16, tag="conv")
                nc.scalar.activation(
                    conv_sb[:], out_ps[:], AF.Identity,
                    accum_out=sum_acc[:, t:t + 1])
                sq_sb = spool.tile([C, TILE_V], BF16, tag="sq")
                nc.scalar.activation(
                    sq_sb[:], conv_sb[:], AF.Square,
                    accum_out=sq_acc[:, t:t + 1])
                nc.sync.dma_start(
                    convT[:, t * TILE_V:(t + 1) * TILE_V], conv_sb[:])

            stats_sb = cpool.tile([C, 2], F32, tag="stats_sb")
            nc.vector.reduce_sum(
                stats_sb[:, 0:1], sum_acc[:], axis=mybir.AxisListType.X)
            nc.vector.reduce_sum(
                stats_sb[:, 1:2], sq_acc[:], axis=mybir.AxisListType.X)
            nc.sync.dma_start(stats[:], stats_sb[:])

    nc.finalize()
    return nc


def build_pass2(n_tiles=N_TILES):
    nc = bacc.Bacc("TRN2", target_bir_lowering=False, debug=False,
                   num_devices=N_CORES)
    convT = nc.dram_tensor("convT", [C, n_tiles * TILE_V], BF16,
                           kind="ExternalInput")
    scale = nc.dram_tensor("scale", [C, 1], F32, kind="ExternalInput")
    shift = nc.dram_tensor("shift", [C, 1], F32, kind="ExternalInput")
    out = nc.dram_tensor("out", [n_tiles * TILE_V, C], F32,
                         kind="ExternalOutput")

    with tile.TileContext(nc) as tc:
        with (
            tc.tile_pool(name="const", bufs=1) as cpool,
            tc.tile_pool(name="sp", bufs=3) as spool,
            tc.tile_pool(name="ps", bufs=2, space="PSUM") as ps,
        ):
            ident = cpool.tile([C, C], F32, tag="ident")
            make_identity(nc, ident[:])
            scale_sb = cpool.tile([C, 1], F32, tag="scale")
            shift_sb = cpool.tile([C, 1], F32, tag="shift")
            nc.sync.dma_start(scale_sb[:], scale[:])
            nc.sync.dma_start(shift_sb[:], shift[:])

            for t in range(n_tiles):
                nsb = spool.tile([C, TILE_V], BF16, tag="nsb")
                nc.sync.dma_start(
                    nsb[:], convT[:, t * TILE_V:(t + 1) * TILE_V])
                nrm = spool.tile([C, TILE_V], F32, tag="nrm")
                nc.scalar.activation(
                    nrm[:], nsb[:], AF.Relu,
                    bias=shift_sb[:], scale=scale_sb[:])
                op = ps.tile([128, BLOCKS * C], F32, tag="op")
                for b in range(BLOCKS):
                    nc.tensor.transpose(
                        op[:, b * C:(b + 1) * C],
                        nrm[:, b * 128:(b + 1) * 128],
                        ident[:],
                    )
                osb = spool.tile([128, BLOCKS * C], F32, tag="osb")
                nc.vector.tensor_copy(osb[:], op[:])
                nc.sync.dma_start(
                    out[t * TILE_V:(t + 1) * TILE_V, :].rearrange(
                        "(b p) c -> p b c", p=128),
                    osb[:].rearrange("p (b c) -> p b c", b=BLOCKS),
                )

    nc.finalize()
    return nc


# --------------------------------------------------------------------------
# reusable PJRT runner (keeps the compiled executable across calls)
# --------------------------------------------------------------------------
class _Runner:
    """Runs a bass NEFF over n_cores devices via shard_map.

    `replicated`: input names fed once (same array on every core).
    Inputs/outputs are jax arrays; sharded inputs are globally concatenated
    on axis 0 (core-major). Outputs stay on device until converted.
    """

    def __init__(self, nc, n_cores, replicated=()):
        import jax
        from jax.sharding import Mesh, PartitionSpec
        from jax.experimental.shard_map import shard_map
        from concourse import bass2jax

        bass2jax.install_neuronx_cc_hook()
        self.jax = jax
        self.n_cores = n_cores
        self.replicated = set(replicated)
        pname = nc.partition_id_tensor.name if nc.partition_id_tensor else None
        in_names, out_names, out_avals, zero_outs = [], [], [], []
        for alloc in nc.m.functions[0].allocations:
            if not isinstance(alloc, mybir.MemoryLocationSet):
                continue
            name = alloc.memorylocations[0].name
            if alloc.kind == "ExternalInput":
                if name != pname:
                    in_names.append(name)
            elif alloc.kind == "ExternalOutput":
                out_names.append(name)
                shape = tuple(alloc.tensor_shape)
                dtype = mybir.dt.np(alloc.dtype)
                out_avals.append(jax.core.ShapedArray(shape, dtype))
                zero_outs.append(np.zeros(shape, dtype))
        self.in_names, self.out_names = in_names, out_names
        self.out_avals, self.zero_outs = out_avals, zero_outs
        n_params = len(in_names)
        self.n_params = n_params
        all_in = list(in_names) + list(out_names)
        if pname is not None:
            all_in.append(pname)

        def _body(*args):
            operands = list(args)
            if pname is not None:
                operands.append(bass2jax.partition_id_tensor())
            outs = bass2jax._bass_exec_p.bind(
                *operands,
                out_avals=tuple(out_avals),
                in_names=tuple(all_in),
                out_names=tuple(out_names),
                lowering_input_output_aliases=(),
                sim_require_finite=True,
                sim_require_nnan=True,
                nc=nc,
            )
            return tuple(outs)

        devices = jax.devices()[:n_cores]
        mesh = Mesh(np.asarray(devices), ("core",))
        in_specs = tuple(
            PartitionSpec() if n in self.replicated else PartitionSpec("core")
            for n in in_names
        ) + (PartitionSpec("core"),) * len(out_names)
        self.fn = jax.jit(
            shard_map(_body, mesh=mesh, in_specs=in_specs,
                      out_specs=(PartitionSpec("core"),) * len(out_names),
                      check_rep=False),
            keep_unused=True,
        )

    def prep(self, in_map):
        """in_map: replicated name -> array; sharded name -> list of per-core
        arrays OR pre-concatenated global array / jax array."""
        args = []
        for n in self.in_names:
            v = in_map[n]
            if isinstance(v, list):
                v = np.concatenate([np.asarray(x) for x in v], axis=0)
            args.append(v)
        args += [
            np.zeros((self.n_cores * z.shape[0], *z.shape[1:]), z.dtype)
            for z in self.zero_outs
        ]
        return args

    def run(self, in_map):
        outs = self.fn(*self.prep(in_map))
        self.jax.block_until_ready(outs)
        return dict(zip(self.out_names, outs))

    def percore(self, arr_global, name):
        i = self.out_names.index(name)
        return np.asarray(arr_global).reshape(
            self.n_cores, *self.out_avals[i].shape)


# --------------------------------------------------------------------------
# host-side glue
# --------------------------------------------------------------------------
def _arrange_idx(nbr_shard: np.ndarray, n_tiles: int) -> np.ndarray:
    """[n_tiles*512, 27] -> [n_tiles, 128, 108] with
    idx_arr[t, p, k*4+b] = nbr[t*512 + b*128 + p, k]."""
    a = nbr_shard.reshape(n_tiles, BLOCKS, 128, KVOL)       # [t, b, p, k]
    a = a.transpose(0, 2, 3, 1)                             # [t, p, k, b]
    return np.ascontiguousarray(a.reshape(n_tiles, 128, J), dtype=np.int32)



def _wrap16(v):
    a = v.reshape(-1, 16).T.astype(np.int16)
    return np.tile(a, (8, 1))


def _prep_gather_idx(nbr_shard, n_tiles):
    l1 = np.zeros((n_tiles, NWIN, 128, WSLOTS // 16), np.int16)
    l2 = np.zeros((n_tiles, 128, NREQ // 16), np.int16)
    nbr = nbr_shard.reshape(n_tiles, TILE_V, KVOL)
    for t in range(n_tiles):
        idx_flat = nbr[t].T.reshape(-1).astype(np.int64)
        w = idx_flat >> 15
        loc = (idx_flat & 32767).astype(np.int16)
        order = np.argsort(w, kind="stable")
        counts = np.bincount(w, minlength=NWIN)
        if counts.max() > WSLOTS:
            raise ValueError("window overflow")
        starts = np.zeros(NWIN, np.int64)
        starts[1:] = np.cumsum(counts)[:-1]
        pos = np.empty(NREQ, np.int64)
        pos[order] = np.arange(NREQ) - starts[w[order]]
        token = (w * WSLOTS + pos).astype(np.int16)
        l2[t] = _wrap16(token)
        sl = loc[order]
        for win in range(NWIN):
            buf = np.zeros(WSLOTS, np.int16)
            buf[:counts[win]] = sl[starts[win]:starts[win] + counts[win]]
            l1[t, win] = _wrap16(buf)
    return l1, l2


def _prep_feats(feats):
    fp = np.zeros((N_TOTAL, CP), np.float32)
    fp[:, :C] = feats
    return fp.astype(BF16_NP)


def _prep_w(W):
    wp = np.zeros((CP, KVOL, C), np.float32)
    wp[:C] = np.asarray(W, np.float32).transpose(1, 0, 2)
    return wp.astype(BF16_NP)


def run_pipeline(feats, nbr, W, gamma, beta, n_tiles):
    n_total = n_tiles * TILE_V * N_CORES
    npc = n_tiles * TILE_V
    key = ("p1", n_tiles)
    if key not in _cache:
        _cache[key] = _Runner(build_pass1_v2(n_tiles), N_CORES,
                              replicated=("featsP", "Wb"))
    r1 = _cache[key]
    key2 = ("p2", n_tiles)
    if key2 not in _cache:
        _cache[key2] = _Runner(build_pass2(n_tiles), N_CORES,
                               replicated=("scale", "shift"))
    r2 = _cache[key2]

    l1_all, l2_all = [], []
    for c in range(N_CORES):
        l1, l2 = _prep_gather_idx(nbr[c * npc:(c + 1) * npc], n_tiles)
        l1_all.append(l1)
        l2_all.append(l2)
    res1 = r1.run({
        "featsP": _prep_feats(np.ascontiguousarray(feats, np.float32)),
        "l1idx": np.concatenate(l1_all, axis=0),
        "l2idx": np.concatenate(l2_all, axis=0),
        "Wb": _prep_w(W)})

    # combine BN stats on host (768 B per core)
    stats = r1.percore(res1["stats"], "stats")         # [8, 96, 2]
    s = stats.sum(axis=0, dtype=np.float64)
    mean = s[:, 0] / n_total
    var = s[:, 1] / n_total - mean * mean
    inv = 1.0 / np.sqrt(var + BN_EPS)
    scale = (gamma.astype(np.float64).reshape(C) * inv).astype(np.float32)
    shift = (beta.astype(np.float64).reshape(C)
             - mean * gamma.astype(np.float64).reshape(C) * inv
             ).astype(np.float32)

    # convT stays device-resident (jax array) between the passes
    res2 = r2.run({
        "convT": res1["convT"],
        "scale": scale.reshape(C, 1),
        "shift": shift.reshape(C, 1),
    })
    return np.asarray(res2["out"])


def kernel(feats, nbr_idx, W, gamma, beta):
    feats = np.ascontiguousarray(feats, dtype=np.float32)
    W = np.ascontiguousarray(W, dtype=np.float32)
    nbr = np.asarray(nbr_idx)
    gamma = np.asarray(gamma, dtype=np.float32)
    beta = np.asarray(beta, dtype=np.float32)
    return run_pipeline(feats, nbr, W, gamma, beta, N_TILES)


def measure_floor():
    """Axon dispatch floor: exec wall of a trivial 1-core kernel."""
    import time
    import jax

    if "floor" not in _cache:
        nc = bacc.Bacc("TRN2", target_bir_lowering=False, debug=False,
                       num_devices=1)
        x = nc.dram_tensor("x", [128, 128], F32, kind="ExternalInput")
        y = nc.dram_tensor("y", [128, 128], F32, kind="ExternalOutput")
        with tile.TileContext(nc) as tc:
            with tc.tile_pool(name="s", bufs=1) as p:
                t = p.tile([128, 128], F32, tag="t")
                nc.sync.dma_start(t[:], x[:])
                nc.vector.tensor_copy(t[:], t[:])
                nc.sync.dma_start(y[:], t[:])
        nc.finalize()
        _cache["floor"] = _Runner(nc, 1)
    r = _cache["floor"]
    args = [jax.device_put(a) for a in
            r.prep({"x": np.ones((128, 128), np.float32)})]
    jax.block_until_ready(args)
    outs = r.fn(*args)
    jax.block_until_ready(outs)
    ts = []
    for _ in range(8):
        t0 = time.perf_counter()
        outs = r.fn(*args)
        jax.block_until_ready(outs)
        ts.append(time.perf_counter() - t0)
    return min(ts)


def measure_exec(feats, nbr_idx, W, n_tiles=N_TILES):
    """Re-execute both compiled passes with device-resident inputs and
    return (pass1_s, pass2_s) minimum wall times — an upper bound on device
    exec time (includes one axon dispatch round-trip each)."""
    import time
    import jax

    r1 = _cache[("p1", n_tiles)]
    r2 = _cache[("p2", n_tiles)]
    npc = n_tiles * TILE_V
    nbr = np.asarray(nbr_idx)
    l1_all, l2_all = [], []
    for c in range(N_CORES):
        l1, l2 = _prep_gather_idx(nbr[c * npc:(c + 1) * npc], n_tiles)
        l1_all.append(l1)
        l2_all.append(l2)
    args1 = [jax.device_put(a) for a in r1.prep({
        "featsP": _prep_feats(np.ascontiguousarray(feats, np.float32)),
        "l1idx": np.concatenate(l1_all, axis=0),
        "l2idx": np.concatenate(l2_all, axis=0),
        "Wb": _prep_w(W)})]
    jax.block_until_ready(args1)
    t1 = []
    for _ in range(4):
        t0 = time.perf_counter()
        outs = r1.fn(*args1)
        jax.block_until_ready(outs)
        t1.append(time.perf_counter() - t0)
    res1 = dict(zip(r1.out_names, outs))
    args2 = r2.prep({
        "convT": res1["convT"],
        "scale": np.ones((C, 1), np.float32),
        "shift": np.zeros((C, 1), np.float32)})
    args2 = [jax.device_put(a) if isinstance(a, np.ndarray) else a
             for a in args2]
    jax.block_until_ready(args2)
    t2 = []
    for _ in range(4):
        t0 = time.perf_counter()
        outs2 = r2.fn(*args2)
        jax.block_until_ready(outs2)
        t2.append(time.perf_counter() - t0)
    return min(t1), min(t2)


# revision 9
# speedup vs baseline: 1.5951x; 1.5951x over previous
"""Trainium2 Bass kernel for nn_BasicConvolutionBlock (sparse conv + BN + ReLU).

Math (per reference):
    conv[n] = sum_k feats[nbr_idx[n, k]] @ W[k]       # [N, 96], k = 0..26
    y = (conv - mean) * rsqrt(var + eps) * gamma + beta ; relu(y)

Distribution: voxel (N) dimension sharded across 8 NeuronCores; feats table
(bf16 — halves the random-gather DMA bytes vs f32; rel err ~4e-3, well under
the 2e-2 gate) and weights replicated per core.

Two NEFFs (the AllReduce-in-kernel path is unstable under the axon PJRT
bridge, so per-core BN partial sums are combined on the host -- 768 B of
float math -- between the two device passes):

  pass 1 (per core, per 512-voxel tile):
    - 108 indirect DMA gathers (128 bf16 rows of 192 B each) -> SBUF
      [128, 108, 96].  One indirect DMA per idx column: multi-column offset
      APs return garbage on HW (the SWDGE ucode only honors [128, 1]
      offsets), so the ~1 us/instruction SWDGE overhead is the pass-1 floor.
    - per group of 3 offsets k: 12 PE transposes (bf16: 1 cyc/row)
      [128v, 96c] -> PSUM [96, 1536] bf16, one PSUM->SBUF copy (alternating
      DVE / scalar engine to balance load), 3 accumulating bf16 matmuls
      W_k.T @ G_k.T -> PSUM [96, 512] f32
    - BN partial sum/sumsq via ACT accum; conv staged to DRAM channel-major
      in bf16
  pass 2: y = relu(conv * scale + shift) (fused ACT op, f32 out), PE
    transpose back to row-major, store f32.
"""
import numpy as np

import concourse.bass as bass
import concourse.bacc as bacc
import concourse.tile as tile
import concourse.mybir as mybir
from concourse.masks import make_identity

F32 = mybir.dt.float32
BF16 = mybir.dt.bfloat16
I32 = mybir.dt.int32
AF = mybir.ActivationFunctionType
BF16_NP = mybir.dt.np(BF16)

N_TOTAL = 262144
C = 96
KVOL = 27
N_CORES = 8
N_PER_CORE = N_TOTAL // N_CORES      # 32768
TILE_V = 512                         # voxels per tile
BLOCKS = TILE_V // 128               # 4
J = KVOL * BLOCKS                    # 108 gathered rows per partition per tile
N_TILES = N_PER_CORE // TILE_V       # 64
BN_EPS = 1e-5
KGRP = 3                             # kernel offsets per transpose/copy group

_cache = {}


# --------------------------------------------------------------------------
# graph builders
# --------------------------------------------------------------------------
def build_pass1(n_tiles=N_TILES, n_cores=N_CORES, gbufs=2, rbufs=3):
    nc = bacc.Bacc("TRN2", target_bir_lowering=False, debug=False,
                   num_devices=n_cores)

    featsb = nc.dram_tensor("featsb", [N_TOTAL, C], BF16,
                            kind="ExternalInput")
    # host-arranged: idx_arr[t, p, k*4+b] = nbr[t*512 + b*128 + p, k]
    idx_arr = nc.dram_tensor("idx_arr", [n_tiles, 128, J], I32,
                             kind="ExternalInput")
    # host-arranged: Wb[ci, k, co] = W[k, ci, co] in bf16
    Wb = nc.dram_tensor("Wb", [C, KVOL, C], BF16, kind="ExternalInput")
    convT = nc.dram_tensor("convT", [C, n_tiles * TILE_V], BF16,
                           kind="ExternalOutput")
    stats = nc.dram_tensor("stats", [C, 2], F32, kind="ExternalOutput")

    with tile.TileContext(nc) as tc:
        with (
            tc.tile_pool(name="const", bufs=1) as cpool,
            tc.tile_pool(name="gp", bufs=gbufs) as gpool,
            tc.tile_pool(name="rp", bufs=rbufs) as rpool,
            tc.tile_pool(name="sp", bufs=2) as spool,
            tc.tile_pool(name="psA", bufs=2, space="PSUM") as psA,
            tc.tile_pool(name="psB", bufs=2, space="PSUM") as psB,
        ):
            ident = cpool.tile([128, 128], BF16, tag="ident")
            make_identity(nc, ident[:])
            w_sb = cpool.tile([C, KVOL, C], BF16, tag="w")
            nc.sync.dma_start(w_sb[:], Wb[:])

            sum_acc = cpool.tile([C, n_tiles], F32, tag="sum_acc")
            sq_acc = cpool.tile([C, n_tiles], F32, tag="sq_acc")

            for t in range(n_tiles):
                idx_sb = gpool.tile([128, J], I32, tag="idx")
                nc.sync.dma_start(idx_sb[:], idx_arr[t, :, :])

                g_sb = gpool.tile([128, J, C], BF16, tag="g")
                # NOTE: one indirect DMA per column — multi-column offset
                # APs return garbage on HW (ucode only honors [128, 1]).
                for j in range(J):
                    nc.gpsimd.indirect_dma_start(
                        out=g_sb[:, j, :],
                        out_offset=None,
                        in_=featsb[:],
                        in_offset=bass.IndirectOffsetOnAxis(
                            ap=idx_sb[:, j:j + 1], axis=0),
                    )

                out_ps = psA.tile([C, TILE_V], F32, tag="outp")
                for gi, k0 in enumerate(range(0, KVOL, KGRP)):
                    tp = psB.tile([C, KGRP * TILE_V], BF16, tag="tp")
                    for dk in range(KGRP):
                        for b in range(BLOCKS):
                            nc.tensor.transpose(
                                tp[:, (dk * BLOCKS + b) * 128:
                                   (dk * BLOCKS + b + 1) * 128],
                                g_sb[:, (k0 + dk) * BLOCKS + b, :],
                                ident[:],
                            )
                    rhs = rpool.tile([C, KGRP * TILE_V], BF16, tag="rhs")
                    if gi % 2 == 0:
                        nc.vector.tensor_copy(rhs[:], tp[:])
                    else:
                        nc.scalar.activation(rhs[:], tp[:], AF.Identity)
                    for dk in range(KGRP):
                        k = k0 + dk
                        nc.tensor.matmul(
                            out_ps[:], w_sb[:, k, :],
                            rhs[:, dk * TILE_V:(dk + 1) * TILE_V],
                            start=(k == 0), stop=(k == KVOL - 1),
                        )

                conv_sb = spool.tile([C, TILE_V], BF16, tag="conv")
                nc.scalar.activation(
                    conv_sb[:], out_ps[:], AF.Identity,
                    accum_out=sum_acc[:, t:t + 1])
                sq_sb = spool.tile([C, TILE_V], BF16, tag="sq")
                nc.scalar.activation(
                    sq_sb[:], conv_sb[:], AF.Square,
                    accum_out=sq_acc[:, t:t + 1])
                nc.sync.dma_start(
                    convT[:, t * TILE_V:(t + 1) * TILE_V], conv_sb[:])

            stats_sb = cpool.tile([C, 2], F32, tag="stats_sb")
            nc.vector.reduce_sum(
                stats_sb[:, 0:1], sum_acc[:], axis=mybir.AxisListType.X)
            nc.vector.reduce_sum(
                stats_sb[:, 1:2], sq_acc[:], axis=mybir.AxisListType.X)
            nc.sync.dma_start(stats[:], stats_sb[:])

    nc.finalize()
    return nc


def build_pass2(n_tiles=N_TILES):
    nc = bacc.Bacc("TRN2", target_bir_lowering=False, debug=False,
                   num_devices=N_CORES)
    convT = nc.dram_tensor("convT", [C, n_tiles * TILE_V], BF16,
                           kind="ExternalInput")
    scale = nc.dram_tensor("scale", [C, 1], F32, kind="ExternalInput")
    shift = nc.dram_tensor("shift", [C, 1], F32, kind="ExternalInput")
    out = nc.dram_tensor("out", [n_tiles * TILE_V, C], F32,
                         kind="ExternalOutput")

    with tile.TileContext(nc) as tc:
        with (
            tc.tile_pool(name="const", bufs=1) as cpool,
            tc.tile_pool(name="sp", bufs=3) as spool,
            tc.tile_pool(name="ps", bufs=2, space="PSUM") as ps,
        ):
            ident = cpool.tile([C, C], F32, tag="ident")
            make_identity(nc, ident[:])
            scale_sb = cpool.tile([C, 1], F32, tag="scale")
            shift_sb = cpool.tile([C, 1], F32, tag="shift")
            nc.sync.dma_start(scale_sb[:], scale[:])
            nc.sync.dma_start(shift_sb[:], shift[:])

            for t in range(n_tiles):
                nsb = spool.tile([C, TILE_V], BF16, tag="nsb")
                nc.sync.dma_start(
                    nsb[:], convT[:, t * TILE_V:(t + 1) * TILE_V])
                nrm = spool.tile([C, TILE_V], F32, tag="nrm")
                nc.scalar.activation(
                    nrm[:], nsb[:], AF.Relu,
                    bias=shift_sb[:], scale=scale_sb[:])
                op = ps.tile([128, BLOCKS * C], F32, tag="op")
                for b in range(BLOCKS):
                    nc.tensor.transpose(
                        op[:, b * C:(b + 1) * C],
                        nrm[:, b * 128:(b + 1) * 128],
                        ident[:],
                    )
                osb = spool.tile([128, BLOCKS * C], F32, tag="osb")
                nc.vector.tensor_copy(osb[:], op[:])
                nc.sync.dma_start(
                    out[t * TILE_V:(t + 1) * TILE_V, :].rearrange(
                        "(b p) c -> p b c", p=128),
                    osb[:].rearrange("p (b c) -> p b c", b=BLOCKS),
                )

    nc.finalize()
    return nc


# --------------------------------------------------------------------------
# reusable PJRT runner (keeps the compiled executable across calls)
# --------------------------------------------------------------------------
class _Runner:
    """Runs a bass NEFF over n_cores devices via shard_map.

    `replicated`: input names fed once (same array on every core).
    Inputs/outputs are jax arrays; sharded inputs are globally concatenated
    on axis 0 (core-major). Outputs stay on device until converted.
    """

    def __init__(self, nc, n_cores, replicated=()):
        import jax
        from jax.sharding import Mesh, PartitionSpec
        from jax.experimental.shard_map import shard_map
        from concourse import bass2jax

        bass2jax.install_neuronx_cc_hook()
        self.jax = jax
        self.n_cores = n_cores
        self.replicated = set(replicated)
        pname = nc.partition_id_tensor.name if nc.partition_id_tensor else None
        in_names, out_names, out_avals, zero_outs = [], [], [], []
        for alloc in nc.m.functions[0].allocations:
            if not isinstance(alloc, mybir.MemoryLocationSet):
                continue
            name = alloc.memorylocations[0].name
            if alloc.kind == "ExternalInput":
                if name != pname:
                    in_names.append(name)
            elif alloc.kind == "ExternalOutput":
                out_names.append(name)
                shape = tuple(alloc.tensor_shape)
                dtype = mybir.dt.np(alloc.dtype)
                out_avals.append(jax.core.ShapedArray(shape, dtype))
                zero_outs.append(np.zeros(shape, dtype))
        self.in_names, self.out_names = in_names, out_names
        self.out_avals, self.zero_outs = out_avals, zero_outs
        n_params = len(in_names)
        self.n_params = n_params
        all_in = list(in_names) + list(out_names)
        if pname is not None:
            all_in.append(pname)

        def _body(*args):
            operands = list(args)
            if pname is not None:
                operands.append(bass2jax.partition_id_tensor())
            outs = bass2jax._bass_exec_p.bind(
                *operands,
                out_avals=tuple(out_avals),
                in_names=tuple(all_in),
                out_names=tuple(out_names),
                lowering_input_output_aliases=(),
                sim_require_finite=True,
                sim_require_nnan=True,
                nc=nc,
            )
            return tuple(outs)

        devices = jax.devices()[:n_cores]
        mesh = Mesh(np.asarray(devices), ("core",))
        in_specs = tuple(
            PartitionSpec() if n in self.replicated else PartitionSpec("core")
            for n in in_names
        ) + (PartitionSpec("core"),) * len(out_names)
        self.fn = jax.jit(
            shard_map(_body, mesh=mesh, in_specs=in_specs,
                      out_specs=(PartitionSpec("core"),) * len(out_names),
                      check_rep=False),
            keep_unused=True,
        )

    def prep(self, in_map):
        """in_map: replicated name -> array; sharded name -> list of per-core
        arrays OR pre-concatenated global array / jax array."""
        args = []
        for n in self.in_names:
            v = in_map[n]
            if isinstance(v, list):
                v = np.concatenate([np.asarray(x) for x in v], axis=0)
            args.append(v)
        args += [
            np.zeros((self.n_cores * z.shape[0], *z.shape[1:]), z.dtype)
            for z in self.zero_outs
        ]
        return args

    def run(self, in_map):
        outs = self.fn(*self.prep(in_map))
        self.jax.block_until_ready(outs)
        return dict(zip(self.out_names, outs))

    def percore(self, arr_global, name):
        i = self.out_names.index(name)
        return np.asarray(arr_global).reshape(
            self.n_cores, *self.out_avals[i].shape)


# --------------------------------------------------------------------------
# host-side glue
# --------------------------------------------------------------------------
def _arrange_idx(nbr_shard: np.ndarray, n_tiles: int) -> np.ndarray:
    """[n_tiles*512, 27] -> [n_tiles, 128, 108] with
    idx_arr[t, p, k*4+b] = nbr[t*512 + b*128 + p, k]."""
    a = nbr_shard.reshape(n_tiles, BLOCKS, 128, KVOL)       # [t, b, p, k]
    a = a.transpose(0, 2, 3, 1)                             # [t, p, k, b]
    return np.ascontiguousarray(a.reshape(n_tiles, 128, J), dtype=np.int32)


def run_pipeline(feats, nbr, W, gamma, beta, n_tiles):
    n_total = n_tiles * TILE_V * N_CORES
    npc = n_tiles * TILE_V
    key = ("p1", n_tiles)
    if key not in _cache:
        _cache[key] = _Runner(build_pass1(n_tiles), N_CORES,
                              replicated=("featsb", "Wb"))
    r1 = _cache[key]
    key2 = ("p2", n_tiles)
    if key2 not in _cache:
        _cache[key2] = _Runner(build_pass2(n_tiles), N_CORES,
                               replicated=("scale", "shift"))
    r2 = _cache[key2]

    idx_all = np.concatenate([
        _arrange_idx(nbr[c * npc:(c + 1) * npc], n_tiles)
        for c in range(N_CORES)
    ], axis=0)
    featsb = np.ascontiguousarray(feats, np.float32).astype(BF16_NP)
    Wb = np.ascontiguousarray(
        np.asarray(W, np.float32).transpose(1, 0, 2)).astype(BF16_NP)
    res1 = r1.run({"featsb": featsb, "idx_arr": idx_all, "Wb": Wb})

    # combine BN stats on host (768 B per core)
    stats = r1.percore(res1["stats"], "stats")         # [8, 96, 2]
    s = stats.sum(axis=0, dtype=np.float64)
    mean = s[:, 0] / n_total
    var = s[:, 1] / n_total - mean * mean
    inv = 1.0 / np.sqrt(var + BN_EPS)
    scale = (gamma.astype(np.float64).reshape(C) * inv).astype(np.float32)
    shift = (beta.astype(np.float64).reshape(C)
             - mean * gamma.astype(np.float64).reshape(C) * inv
             ).astype(np.float32)

    # convT stays device-resident (jax array) between the passes
    res2 = r2.run({
        "convT": res1["convT"],
        "scale": scale.reshape(C, 1),
        "shift": shift.reshape(C, 1),
    })
    return np.asarray(res2["out"])


def kernel(feats, nbr_idx, W, gamma, beta):
    feats = np.ascontiguousarray(feats, dtype=np.float32)
    W = np.ascontiguousarray(W, dtype=np.float32)
    nbr = np.asarray(nbr_idx)
    gamma = np.asarray(gamma, dtype=np.float32)
    beta = np.asarray(beta, dtype=np.float32)
    return run_pipeline(feats, nbr, W, gamma, beta, N_TILES)


def measure_floor():
    """Axon dispatch floor: exec wall of a trivial 1-core kernel."""
    import time
    import jax

    if "floor" not in _cache:
        nc = bacc.Bacc("TRN2", target_bir_lowering=False, debug=False,
                       num_devices=1)
        x = nc.dram_tensor("x", [128, 128], F32, kind="ExternalInput")
        y = nc.dram_tensor("y", [128, 128], F32, kind="ExternalOutput")
        with tile.TileContext(nc) as tc:
            with tc.tile_pool(name="s", bufs=1) as p:
                t = p.tile([128, 128], F32, tag="t")
                nc.sync.dma_start(t[:], x[:])
                nc.vector.tensor_copy(t[:], t[:])
                nc.sync.dma_start(y[:], t[:])
        nc.finalize()
        _cache["floor"] = _Runner(nc, 1)
    r = _cache["floor"]
    args = [jax.device_put(a) for a in
            r.prep({"x": np.ones((128, 128), np.float32)})]
    jax.block_until_ready(args)
    outs = r.fn(*args)
    jax.block_until_ready(outs)
    ts = []
    for _ in range(8):
        t0 = time.perf_counter()
        outs = r.fn(*args)
        jax.block_until_ready(outs)
        ts.append(time.perf_counter() - t0)
    return min(ts)


def measure_exec(feats, nbr_idx, W, n_tiles=N_TILES):
    """Re-execute both compiled passes with device-resident inputs and
    return (pass1_s, pass2_s) minimum wall times — an upper bound on device
    exec time (includes one axon dispatch round-trip each)."""
    import time
    import jax

    r1 = _cache[("p1", n_tiles)]
    r2 = _cache[("p2", n_tiles)]
    npc = n_tiles * TILE_V
    nbr = np.asarray(nbr_idx)
    idx_all = np.concatenate([
        _arrange_idx(nbr[c * npc:(c + 1) * npc], n_tiles)
        for c in range(N_CORES)
    ], axis=0)
    featsb = np.ascontiguousarray(feats, np.float32).astype(BF16_NP)
    Wb = np.ascontiguousarray(
        np.asarray(W, np.float32).transpose(1, 0, 2)).astype(BF16_NP)
    args1 = [jax.device_put(a) for a in r1.prep({
        "featsb": featsb,
        "idx_arr": idx_all,
        "Wb": Wb})]
    jax.block_until_ready(args1)
    t1 = []
    for _ in range(4):
        t0 = time.perf_counter()
        outs = r1.fn(*args1)
        jax.block_until_ready(outs)
        t1.append(time.perf_counter() - t0)
    res1 = dict(zip(r1.out_names, outs))
    args2 = r2.prep({
        "convT": res1["convT"],
        "scale": np.ones((C, 1), np.float32),
        "shift": np.zeros((C, 1), np.float32)})
    args2 = [jax.device_put(a) if isinstance(a, np.ndarray) else a
             for a in args2]
    jax.block_until_ready(args2)
    t2 = []
    for _ in range(4):
        t0 = time.perf_counter()
        outs2 = r2.fn(*args2)
        jax.block_until_ready(outs2)
        t2.append(time.perf_counter() - t0)
    return min(t1), min(t2)


# revision 11
# speedup vs baseline: 1.8004x; 1.1287x over previous
"""Trainium2 Bass kernel for nn_BasicConvolutionBlock (sparse conv + BN + ReLU).

Math (per reference):
    conv[n] = sum_k feats[nbr_idx[n, k]] @ W[k]       # [N, 96], k = 0..26
    y = (conv - mean) * rsqrt(var + eps) * gamma + beta ; relu(y)

Distribution: voxel (N) dimension sharded across 8 NeuronCores; feats table
(bf16 — halves the random-gather DMA bytes vs f32; rel err ~4e-3, well under
the 2e-2 gate) and weights replicated per core.

Two NEFFs (the AllReduce-in-kernel path is unstable under the axon PJRT
bridge, so per-core BN partial sums are combined on the host -- 768 B of
float math -- between the two device passes):

  pass 1 (per core, per 512-voxel tile):
    - 108 indirect DMA gathers (128 bf16 rows of 192 B each) -> SBUF
      [128, 108, 96].  One indirect DMA per idx column: multi-column offset
      APs return garbage on HW (the SWDGE ucode only honors [128, 1]
      offsets), so the ~1 us/instruction SWDGE overhead is the pass-1 floor.
    - per group of 3 offsets k: 12 PE transposes (bf16: 1 cyc/row)
      [128v, 96c] -> PSUM [96, 1536] bf16, one PSUM->SBUF copy (alternating
      DVE / scalar engine to balance load), 3 accumulating bf16 matmuls
      W_k.T @ G_k.T -> PSUM [96, 512] f32
    - BN partial sum/sumsq via ACT accum; conv staged to DRAM channel-major
      in bf16
  pass 2: y = relu(conv * scale + shift) (fused ACT op, f32 out), PE
    transpose back to row-major, store f32.
"""
import numpy as np

import concourse.bass as bass
import concourse.bacc as bacc
import concourse.tile as tile
import concourse.mybir as mybir
from concourse.masks import make_identity

F32 = mybir.dt.float32
BF16 = mybir.dt.bfloat16
I32 = mybir.dt.int32
AF = mybir.ActivationFunctionType
BF16_NP = mybir.dt.np(BF16)

N_TOTAL = 262144
C = 96
KVOL = 27
N_CORES = 8
N_PER_CORE = N_TOTAL // N_CORES      # 32768
TILE_V = 512                         # voxels per tile
BLOCKS = TILE_V // 128               # 4
J = KVOL * BLOCKS                    # 108 gathered rows per partition per tile
N_TILES = N_PER_CORE // TILE_V       # 64
BN_EPS = 1e-5
KGRP = 3                             # kernel offsets per transpose/copy group

_cache = {}


# --------------------------------------------------------------------------
# graph builders
# --------------------------------------------------------------------------
def build_pass1(n_tiles=N_TILES, n_cores=N_CORES, gbufs=2, rbufs=3,
                nqueues=2):
    nc = bacc.Bacc("TRN2", target_bir_lowering=False, debug=False,
                   num_devices=n_cores, num_swdge_queues=nqueues)

    featsb = nc.dram_tensor("featsb", [N_TOTAL, C], BF16,
                            kind="ExternalInput")
    # host-arranged: idx_arr[t, p, k*4+b] = nbr[t*512 + b*128 + p, k]
    idx_arr = nc.dram_tensor("idx_arr", [n_tiles, 128, J], I32,
                             kind="ExternalInput")
    # host-arranged: Wb[ci, k, co] = W[k, ci, co] in bf16
    Wb = nc.dram_tensor("Wb", [C, KVOL, C], BF16, kind="ExternalInput")
    convT = nc.dram_tensor("convT", [C, n_tiles * TILE_V], BF16,
                           kind="ExternalOutput")
    stats = nc.dram_tensor("stats", [C, 2], F32, kind="ExternalOutput")

    with tile.TileContext(nc) as tc:
        with (
            tc.tile_pool(name="const", bufs=1) as cpool,
            tc.tile_pool(name="gp", bufs=gbufs) as gpool,
            tc.tile_pool(name="rp", bufs=rbufs) as rpool,
            tc.tile_pool(name="sp", bufs=2) as spool,
            tc.tile_pool(name="psA", bufs=2, space="PSUM") as psA,
            tc.tile_pool(name="psB", bufs=2, space="PSUM") as psB,
        ):
            ident = cpool.tile([128, 128], BF16, tag="ident")
            make_identity(nc, ident[:])
            w_sb = cpool.tile([C, KVOL, C], BF16, tag="w")
            nc.sync.dma_start(w_sb[:], Wb[:])

            sum_acc = cpool.tile([C, n_tiles], F32, tag="sum_acc")
            sq_acc = cpool.tile([C, n_tiles], F32, tag="sq_acc")

            for t in range(n_tiles):
                idx_sb = gpool.tile([128, J], I32, tag="idx")
                nc.sync.dma_start(idx_sb[:], idx_arr[t, :, :])

                g_sb = gpool.tile([128, J, C], BF16, tag="g")
                # NOTE: one indirect DMA per column — multi-column offset
                # APs return garbage on HW (ucode only honors [128, 1]).
                for j in range(J):
                    inst = nc.gpsimd.indirect_dma_start(
                        out=g_sb[:, j, :],
                        out_offset=None,
                        in_=featsb[:],
                        in_offset=bass.IndirectOffsetOnAxis(
                            ap=idx_sb[:, j:j + 1], axis=0),
                    )
                    # spread SWDGE descriptor generation across the dynamic
                    # queues: per-instruction gen (~1 us) is the pass-1
                    # serialization floor on a single queue.
                    q = j % nqueues
                    if q:
                        inst.ins.queue = f"qPoolDynamic{q}"

                out_ps = psA.tile([C, TILE_V], F32, tag="outp")
                for gi, k0 in enumerate(range(0, KVOL, KGRP)):
                    tp = psB.tile([C, KGRP * TILE_V], BF16, tag="tp")
                    for dk in range(KGRP):
                        for b in range(BLOCKS):
                            nc.tensor.transpose(
                                tp[:, (dk * BLOCKS + b) * 128:
                                   (dk * BLOCKS + b + 1) * 128],
                                g_sb[:, (k0 + dk) * BLOCKS + b, :],
                                ident[:],
                            )
                    rhs = rpool.tile([C, KGRP * TILE_V], BF16, tag="rhs")
                    if gi % 2 == 0:
                        nc.vector.tensor_copy(rhs[:], tp[:])
                    else:
                        nc.scalar.activation(rhs[:], tp[:], AF.Identity)
                    for dk in range(KGRP):
                        k = k0 + dk
                        nc.tensor.matmul(
                            out_ps[:], w_sb[:, k, :],
                            rhs[:, dk * TILE_V:(dk + 1) * TILE_V],
                            start=(k == 0), stop=(k == KVOL - 1),
                        )

                conv_sb = spool.tile([C, TILE_V], BF16, tag="conv")
                nc.scalar.activation(
                    conv_sb[:], out_ps[:], AF.Identity,
                    accum_out=sum_acc[:, t:t + 1])
                sq_sb = spool.tile([C, TILE_V], BF16, tag="sq")
                nc.scalar.activation(
                    sq_sb[:], conv_sb[:], AF.Square,
                    accum_out=sq_acc[:, t:t + 1])
                nc.sync.dma_start(
                    convT[:, t * TILE_V:(t + 1) * TILE_V], conv_sb[:])

            stats_sb = cpool.tile([C, 2], F32, tag="stats_sb")
            nc.vector.reduce_sum(
                stats_sb[:, 0:1], sum_acc[:], axis=mybir.AxisListType.X)
            nc.vector.reduce_sum(
                stats_sb[:, 1:2], sq_acc[:], axis=mybir.AxisListType.X)
            nc.sync.dma_start(stats[:], stats_sb[:])

    nc.finalize()
    return nc


def build_pass2(n_tiles=N_TILES):
    nc = bacc.Bacc("TRN2", target_bir_lowering=False, debug=False,
                   num_devices=N_CORES)
    convT = nc.dram_tensor("convT", [C, n_tiles * TILE_V], BF16,
                           kind="ExternalInput")
    scale = nc.dram_tensor("scale", [C, 1], F32, kind="ExternalInput")
    shift = nc.dram_tensor("shift", [C, 1], F32, kind="ExternalInput")
    out = nc.dram_tensor("out", [n_tiles * TILE_V, C], F32,
                         kind="ExternalOutput")

    with tile.TileContext(nc) as tc:
        with (
            tc.tile_pool(name="const", bufs=1) as cpool,
            tc.tile_pool(name="sp", bufs=3) as spool,
            tc.tile_pool(name="ps", bufs=2, space="PSUM") as ps,
        ):
            ident = cpool.tile([C, C], F32, tag="ident")
            make_identity(nc, ident[:])
            scale_sb = cpool.tile([C, 1], F32, tag="scale")
            shift_sb = cpool.tile([C, 1], F32, tag="shift")
            nc.sync.dma_start(scale_sb[:], scale[:])
            nc.sync.dma_start(shift_sb[:], shift[:])

            for t in range(n_tiles):
                nsb = spool.tile([C, TILE_V], BF16, tag="nsb")
                nc.sync.dma_start(
                    nsb[:], convT[:, t * TILE_V:(t + 1) * TILE_V])
                nrm = spool.tile([C, TILE_V], F32, tag="nrm")
                nc.scalar.activation(
                    nrm[:], nsb[:], AF.Relu,
                    bias=shift_sb[:], scale=scale_sb[:])
                op = ps.tile([128, BLOCKS * C], F32, tag="op")
                for b in range(BLOCKS):
                    nc.tensor.transpose(
                        op[:, b * C:(b + 1) * C],
                        nrm[:, b * 128:(b + 1) * 128],
                        ident[:],
                    )
                osb = spool.tile([128, BLOCKS * C], F32, tag="osb")
                nc.vector.tensor_copy(osb[:], op[:])
                nc.sync.dma_start(
                    out[t * TILE_V:(t + 1) * TILE_V, :].rearrange(
                        "(b p) c -> p b c", p=128),
                    osb[:].rearrange("p (b c) -> p b c", b=BLOCKS),
                )

    nc.finalize()
    return nc


# --------------------------------------------------------------------------
# reusable PJRT runner (keeps the compiled executable across calls)
# --------------------------------------------------------------------------
class _Runner:
    """Runs a bass NEFF over n_cores devices via shard_map.

    `replicated`: input names fed once (same array on every core).
    Inputs/outputs are jax arrays; sharded inputs are globally concatenated
    on axis 0 (core-major). Outputs stay on device until converted.
    """

    def __init__(self, nc, n_cores, replicated=()):
        import jax
        from jax.sharding import Mesh, PartitionSpec
        from jax.experimental.shard_map import shard_map
        from concourse import bass2jax

        bass2jax.install_neuronx_cc_hook()
        self.jax = jax
        self.n_cores = n_cores
        self.replicated = set(replicated)
        pname = nc.partition_id_tensor.name if nc.partition_id_tensor else None
        in_names, out_names, out_avals, zero_outs = [], [], [], []
        for alloc in nc.m.functions[0].allocations:
            if not isinstance(alloc, mybir.MemoryLocationSet):
                continue
            name = alloc.memorylocations[0].name
            if alloc.kind == "ExternalInput":
                if name != pname:
                    in_names.append(name)
            elif alloc.kind == "ExternalOutput":
                out_names.append(name)
                shape = tuple(alloc.tensor_shape)
                dtype = mybir.dt.np(alloc.dtype)
                out_avals.append(jax.core.ShapedArray(shape, dtype))
                zero_outs.append(np.zeros(shape, dtype))
        self.in_names, self.out_names = in_names, out_names
        self.out_avals, self.zero_outs = out_avals, zero_outs
        n_params = len(in_names)
        self.n_params = n_params
        all_in = list(in_names) + list(out_names)
        if pname is not None:
            all_in.append(pname)

        def _body(*args):
            operands = list(args)
            if pname is not None:
                operands.append(bass2jax.partition_id_tensor())
            outs = bass2jax._bass_exec_p.bind(
                *operands,
                out_avals=tuple(out_avals),
                in_names=tuple(all_in),
                out_names=tuple(out_names),
                lowering_input_output_aliases=(),
                sim_require_finite=True,
                sim_require_nnan=True,
                nc=nc,
            )
            return tuple(outs)

        devices = jax.devices()[:n_cores]
        mesh = Mesh(np.asarray(devices), ("core",))
        in_specs = tuple(
            PartitionSpec() if n in self.replicated else PartitionSpec("core")
            for n in in_names
        ) + (PartitionSpec("core"),) * len(out_names)
        self.fn = jax.jit(
            shard_map(_body, mesh=mesh, in_specs=in_specs,
                      out_specs=(PartitionSpec("core"),) * len(out_names),
                      check_rep=False),
            keep_unused=True,
        )

    def prep(self, in_map):
        """in_map: replicated name -> array; sharded name -> list of per-core
        arrays OR pre-concatenated global array / jax array."""
        args = []
        for n in self.in_names:
            v = in_map[n]
            if isinstance(v, list):
                v = np.concatenate([np.asarray(x) for x in v], axis=0)
            args.append(v)
        args += [
            np.zeros((self.n_cores * z.shape[0], *z.shape[1:]), z.dtype)
            for z in self.zero_outs
        ]
        return args

    def run(self, in_map):
        outs = self.fn(*self.prep(in_map))
        self.jax.block_until_ready(outs)
        return dict(zip(self.out_names, outs))

    def percore(self, arr_global, name):
        i = self.out_names.index(name)
        return np.asarray(arr_global).reshape(
            self.n_cores, *self.out_avals[i].shape)


# --------------------------------------------------------------------------
# host-side glue
# --------------------------------------------------------------------------
def _arrange_idx(nbr_shard: np.ndarray, n_tiles: int) -> np.ndarray:
    """[n_tiles*512, 27] -> [n_tiles, 128, 108] with
    idx_arr[t, p, k*4+b] = nbr[t*512 + b*128 + p, k]."""
    a = nbr_shard.reshape(n_tiles, BLOCKS, 128, KVOL)       # [t, b, p, k]
    a = a.transpose(0, 2, 3, 1)                             # [t, p, k, b]
    return np.ascontiguousarray(a.reshape(n_tiles, 128, J), dtype=np.int32)


def run_pipeline(feats, nbr, W, gamma, beta, n_tiles):
    n_total = n_tiles * TILE_V * N_CORES
    npc = n_tiles * TILE_V
    key = ("p1", n_tiles)
    if key not in _cache:
        _cache[key] = _Runner(build_pass1(n_tiles), N_CORES,
                              replicated=("featsb", "Wb"))
    r1 = _cache[key]
    key2 = ("p2", n_tiles)
    if key2 not in _cache:
        _cache[key2] = _Runner(build_pass2(n_tiles), N_CORES,
                               replicated=("scale", "shift"))
    r2 = _cache[key2]

    idx_all = np.concatenate([
        _arrange_idx(nbr[c * npc:(c + 1) * npc], n_tiles)
        for c in range(N_CORES)
    ], axis=0)
    featsb = np.ascontiguousarray(feats, np.float32).astype(BF16_NP)
    Wb = np.ascontiguousarray(
        np.asarray(W, np.float32).transpose(1, 0, 2)).astype(BF16_NP)
    res1 = r1.run({"featsb": featsb, "idx_arr": idx_all, "Wb": Wb})

    # combine BN stats on host (768 B per core)
    stats = r1.percore(res1["stats"], "stats")         # [8, 96, 2]
    s = stats.sum(axis=0, dtype=np.float64)
    mean = s[:, 0] / n_total
    var = s[:, 1] / n_total - mean * mean
    inv = 1.0 / np.sqrt(var + BN_EPS)
    scale = (gamma.astype(np.float64).reshape(C) * inv).astype(np.float32)
    shift = (beta.astype(np.float64).reshape(C)
             - mean * gamma.astype(np.float64).reshape(C) * inv
             ).astype(np.float32)

    # convT stays device-resident (jax array) between the passes
    res2 = r2.run({
        "convT": res1["convT"],
        "scale": scale.reshape(C, 1),
        "shift": shift.reshape(C, 1),
    })
    return np.asarray(res2["out"])


def kernel(feats, nbr_idx, W, gamma, beta):
    feats = np.ascontiguousarray(feats, dtype=np.float32)
    W = np.ascontiguousarray(W, dtype=np.float32)
    nbr = np.asarray(nbr_idx)
    gamma = np.asarray(gamma, dtype=np.float32)
    beta = np.asarray(beta, dtype=np.float32)
    return run_pipeline(feats, nbr, W, gamma, beta, N_TILES)


def measure_floor():
    """Axon dispatch floor: exec wall of a trivial 1-core kernel."""
    import time
    import jax

    if "floor" not in _cache:
        nc = bacc.Bacc("TRN2", target_bir_lowering=False, debug=False,
                       num_devices=1)
        x = nc.dram_tensor("x", [128, 128], F32, kind="ExternalInput")
        y = nc.dram_tensor("y", [128, 128], F32, kind="ExternalOutput")
        with tile.TileContext(nc) as tc:
            with tc.tile_pool(name="s", bufs=1) as p:
                t = p.tile([128, 128], F32, tag="t")
                nc.sync.dma_start(t[:], x[:])
                nc.vector.tensor_copy(t[:], t[:])
                nc.sync.dma_start(y[:], t[:])
        nc.finalize()
        _cache["floor"] = _Runner(nc, 1)
    r = _cache["floor"]
    args = [jax.device_put(a) for a in
            r.prep({"x": np.ones((128, 128), np.float32)})]
    jax.block_until_ready(args)
    outs = r.fn(*args)
    jax.block_until_ready(outs)
    ts = []
    for _ in range(8):
        t0 = time.perf_counter()
        outs = r.fn(*args)
        jax.block_until_ready(outs)
        ts.append(time.perf_counter() - t0)
    return min(ts)


def measure_exec(feats, nbr_idx, W, n_tiles=N_TILES):
    """Re-execute both compiled passes with device-resident inputs and
    return (pass1_s, pass2_s) minimum wall times — an upper bound on device
    exec time (includes one axon dispatch round-trip each)."""
    import time
    import jax

    r1 = _cache[("p1", n_tiles)]
    r2 = _cache[("p2", n_tiles)]
    npc = n_tiles * TILE_V
    nbr = np.asarray(nbr_idx)
    idx_all = np.concatenate([
        _arrange_idx(nbr[c * npc:(c + 1) * npc], n_tiles)
        for c in range(N_CORES)
    ], axis=0)
    featsb = np.ascontiguousarray(feats, np.float32).astype(BF16_NP)
    Wb = np.ascontiguousarray(
        np.asarray(W, np.float32).transpose(1, 0, 2)).astype(BF16_NP)
    args1 = [jax.device_put(a) for a in r1.prep({
        "featsb": featsb,
        "idx_arr": idx_all,
        "Wb": Wb})]
    jax.block_until_ready(args1)
    t1 = []
    for _ in range(4):
        t0 = time.perf_counter()
        outs = r1.fn(*args1)
        jax.block_until_ready(outs)
        t1.append(time.perf_counter() - t0)
    res1 = dict(zip(r1.out_names, outs))
    args2 = r2.prep({
        "convT": res1["convT"],
        "scale": np.ones((C, 1), np.float32),
        "shift": np.zeros((C, 1), np.float32)})
    args2 = [jax.device_put(a) if isinstance(a, np.ndarray) else a
             for a in args2]
    jax.block_until_ready(args2)
    t2 = []
    for _ in range(4):
        t0 = time.perf_counter()
        outs2 = r2.fn(*args2)
        jax.block_until_ready(outs2)
        t2.append(time.perf_counter() - t0)
    return min(t1), min(t2)
